# revision 25
# baseline (speedup 1.0000x reference)
"""AttentionCritic Trainium2 kernel.

Problem (hardcoded): A=8 agents, B=8192 batch, S=128 state, ADIM=16 act,
H=512 hid, HEADS=4, D=128. 8 NeuronCores, batch-sharded (1024 batch/core).

Pipeline per core (b = 1024 local batch, chunks of 128):
  Phase A (chunk-outer, agent-inner):
    LN(states), LN([states|actions]) in fp32, normalized out in fp16;
    PE-transpose normalized inputs; fp16 matmuls for s_enc / sa_enc
    (batch-major out via activation-as-stationary); PE-transpose sa_enc;
    keys/vals/sel projections (all heads at once); attention middle on DVE
    (broadcast-AP mul + segmented reduce for logits, tensor_scalar +
    tree-add for weighted values); Prelu evictions on ACT. s_enc and
    attended values bounce through DRAM in fp16.
  Phase B (agent-outer, 512-batch superchunks):
    LN(cin) stats in batch-major, normalize, PE-transpose, critic matmuls
    (h1 feature-major so bc1 folds into the Prelu eviction), all_qs out.
  Host: shard/unshard, fp16 weight casts, argmax-gather of all_qs.
"""
import sys

sys.path.insert(0, "/opt/trn_rl_repo")
from contextlib import ExitStack

import numpy as np

import concourse.bass as bass
import concourse.bacc as bacc
import concourse.mybir as mybir
from concourse import tile
from concourse.bass_utils import run_bass_kernel_spmd
from concourse.masks import make_identity

A, B, S, ADIM = 8, 8192, 128, 16
H, HEADS = 512, 4
D = H // HEADS
EPS = 1e-5
NCORES = 8
BL = B // NCORES          # local batch per core = 1024
P = 128                   # partition size
NCH = BL // P             # chunks per core = 8
NSC = BL // 512           # 512-batch superchunks = 2
SA = S + ADIM             # 144

f32 = mybir.dt.float32
f16 = mybir.dt.float16
FP = mybir.AluOpType
AF = mybir.ActivationFunctionType
AX = mybir.AxisListType
LRELU_SLOPE = 0.01
INV_SQRT_D = 1.0 / float(np.sqrt(D))


def _bc(ap, axis, n):
    """Insert a broadcast (step 0, count n) free dim at position `axis`
    (0 = first free dim)."""
    dims = [list(d) for d in ap.ap]
    dims.insert(1 + axis, [0, n])
    return bass.AP(ap.tensor, ap.offset, dims)


def _bc_front(ap, n):
    """Prepend a broadcast dim (for DRAM->SBUF partition replication)."""
    dims = [[0, n]] + [list(d) for d in ap.ap]
    return bass.AP(ap.tensor, ap.offset, dims)


def _patch_act_tables():
    """Restrict bacc's activation-table choices to the one set that covers
    every func we use (exp, ln, square, parametric_relu, identity, copy) so
    no ACT_TABLE_LOAD churn happens mid-kernel."""
    from concourse import hw_specs
    import concourse.bacc as _bacc

    orig = hw_specs.get_activation_tables

    def only_combined(arch):
        t = orig(arch)
        if "natural_log_exp_and_others" not in t:
            return t
        # keep ordering (act_func_set_id indexes the real act_info.json) but
        # make every other set uncoverable so the chooser sticks to one table
        return {
            k: (v if k == "natural_log_exp_and_others" else set())
            for k, v in t.items()
        }

    only_combined.__wrapped__ = orig
    hw_specs.get_activation_tables = only_combined
    _bacc.get_activation_tables = only_combined


import os
_SKIP = set(os.environ.get("KSKIP", "").split(","))


def build(nonzero_bias):
    if "tables" not in _SKIP:
        _patch_act_tables()
    nc = bacc.Bacc("TRN2", target_bir_lowering=False, debug=False)

    # ---- DRAM I/O ----
    st_d = nc.dram_tensor("states", [A, BL, S], f32, kind="ExternalInput")
    ac_d = nc.dram_tensor("actions", [A, BL, ADIM], f32, kind="ExternalInput")
    ws_s_d = nc.dram_tensor("ws_s", [A, S, H], f16, kind="ExternalInput")
    ws_sa_d = nc.dram_tensor("ws_sa", [A, SA, H], f16, kind="ExternalInput")
    wk_d = nc.dram_tensor("wk", [H, H], f16, kind="ExternalInput")   # [h, (head,d)]
    wv_d = nc.dram_tensor("wv", [H, H], f16, kind="ExternalInput")
    wsel_d = nc.dram_tensor("wsel", [H, H], f16, kind="ExternalInput")
    wc1_d = nc.dram_tensor("wc1", [A, 2 * H, H], f16, kind="ExternalInput")
    wc2_d = nc.dram_tensor("wc2", [A, H, ADIM], f16, kind="ExternalInput")
    # biases (replicated across partitions host-side where needed)
    bs_s_d = nc.dram_tensor("bs_s", [A, H], f16, kind="ExternalInput")
    bs_sa_d = nc.dram_tensor("bs_sa", [A, H], f16, kind="ExternalInput")
    bsel_d = nc.dram_tensor("bsel", [H], f16, kind="ExternalInput")
    bc1_d = nc.dram_tensor("bc1", [A, H], f32, kind="ExternalInput")
    bc2_d = nc.dram_tensor("bc2", [A, ADIM], f32, kind="ExternalInput")

    allqs_d = nc.dram_tensor("allqs", [A, ADIM, BL], f32, kind="ExternalOutput")

    # DRAM scratch (internal)
    senc_d = nc.dram_tensor("senc_scr", [A, BL, H], f16)
    # logits bounce buffer for the diagonal gather (double-buffered by chunk)
    ebig_d = nc.dram_tensor("ebig_scr", [2, HEADS, P, 8 * 16 * A], f16)
    resident_values = False
    vals_d = None if resident_values else nc.dram_tensor("vals_scr", [A, BL, H], f16)

    with tile.TileContext(nc) as tc, ExitStack() as ctx:
        const = ctx.enter_context(tc.tile_pool(name="const", bufs=1))
        wpool = ctx.enter_context(tc.tile_pool(name="weights", bufs=1))
        io = ctx.enter_context(tc.tile_pool(name="io", bufs=2))
        scr = ctx.enter_context(tc.tile_pool(name="scr", bufs=1))
        work = ctx.enter_context(tc.tile_pool(name="work", bufs=2))
        chunkp = ctx.enter_context(tc.tile_pool(name="chunk", bufs=2))
        attn = ctx.enter_context(tc.tile_pool(name="attn", bufs=2))
        ps = ctx.enter_context(tc.tile_pool(name="ps", bufs=2, space="PSUM"))
        psl = ctx.enter_context(tc.tile_pool(name="psl", bufs=2, space="PSUM"))

        ident = const.tile([P, P], f16)
        make_identity(nc, ident[:])
        eps_t = const.tile([P, 1], f32)
        nc.vector.memset(eps_t[:], EPS)
        zero16 = const.tile([P, 1], f16)
        nc.vector.memset(zero16[:], 0.0)

        # ---- resident weights ----
        # w_ss: [S=128 partitions, (a, h)] ; per-agent rhs slice [128, 512]
        w_ss = wpool.tile([P, A * H], f16, tag="w_ss")
        nc.sync.dma_start(w_ss[:].rearrange("p (a h) -> p a h", a=A), ws_s_d[:].rearrange("a s h -> s a h"))
        # w_ssa split: state rows [128, (a,h)], action rows [16, (a,h)]
        w_ssa = wpool.tile([P, A * H], f16, tag="w_ssa")
        nc.sync.dma_start(w_ssa[:].rearrange("p (a h) -> p a h", a=A), ws_sa_d[:, :S, :].rearrange("a s h -> s a h"))
        w_ssa_a = wpool.tile([ADIM, A * H], f16, tag="w_ssa_a")
        nc.sync.dma_start(w_ssa_a[:].rearrange("p (a h) -> p a h", a=A), ws_sa_d[:, S:, :].rearrange("a s h -> s a h"))
        # w_k/v/sel: [h=512 -> 4 tiles of 128, (head,d)=512]
        w_k = wpool.tile([P, 4 * H], f16, tag="w_k")
        nc.sync.dma_start(w_k[:].rearrange("p (t n) -> p t n", t=4), wk_d[:].rearrange("(t p) n -> p t n", p=P))
        w_v = wpool.tile([P, 4 * H], f16, tag="w_v")
        nc.sync.dma_start(w_v[:].rearrange("p (t n) -> p t n", t=4), wv_d[:].rearrange("(t p) n -> p t n", p=P))
        w_sel = wpool.tile([P, 4 * H], f16, tag="w_sel")
        nc.sync.dma_start(w_sel[:].rearrange("p (t n) -> p t n", t=4), wsel_d[:].rearrange("(t p) n -> p t n", p=P))

        if nonzero_bias:
            bias_ss = wpool.tile([P, A * H], f16, tag="b_ss")
            nc.sync.dma_start(bias_ss[:], _bc_front(bs_s_d[:].rearrange("a h -> (a h)"), P))
            bias_ssa = wpool.tile([P, A * H], f16, tag="b_ssa")
            nc.sync.dma_start(bias_ssa[:], _bc_front(bs_sa_d[:].rearrange("a h -> (a h)"), P))
            bias_sel = wpool.tile([P, H], f16, tag="b_sel")
            nc.sync.dma_start(bias_sel[:], _bc_front(bsel_d[:], P))
        bias_c1 = wpool.tile([P, A * 4], f32, tag="b_c1")  # feature-major [128h x (a,ht)]
        nc.sync.dma_start(
            bias_c1[:].rearrange("p (a ht) -> p a ht", a=A),
            bc1_d[:].rearrange("a (ht p) -> p a ht", p=P),
        )
        bias_c2 = wpool.tile([ADIM, A], f32, tag="b_c2")
        nc.sync.dma_start(bias_c2[:], bc2_d[:].rearrange("a o -> o a"))

        inv_s = 1.0 / S
        inv_sa = 1.0 / SA
        inv_2h = 1.0 / (2 * H)
        sxA = wpool.tile([P, A * NCH], f32, tag="sxA")  # sum_h s_enc per (a, chunk)
        sesq = wpool.tile([P, A * NCH], f32, tag="sesq")   # sum_h s_enc^2
        vsum = wpool.tile([P, A * NCH], f32, tag="vsum")   # sum_h values
        vasq = wpool.tile([P, A * NCH], f32, tag="vasq")   # sum_h values^2

        def emit_phase_b(sc):
            for a in range(A):
                wc1 = scr.tile([P, 8 * H], f16, tag="scaled")  # [128f x 8ft, 512h]
                nc.sync.dma_start(
                wc1[:].rearrange("p (ft h) -> p ft h", ft=8),
                wc1_d[a, :, :].rearrange("(ft p) h -> p ft h", p=P),
                )
                wc2 = scr.tile([P, 4 * ADIM], f16, tag="t2")  # [128h x 4ht, 16]
                nc.sync.dma_start(
                wc2[:].rearrange("p (ht o) -> p ht o", ht=4),
                wc2_d[a, :, :].rearrange("(ht p) o -> p ht o", p=P),
                )
                wc13 = wc1[:].rearrange("p (ft h) -> p ft h", ft=8)
                wc23 = wc2[:].rearrange("p (ht o) -> p ht o", ht=4)
                sb0 = sc * 512
                se_b = chunkp.tile([P, 4 * H], f16, tag="sel_all")  # [128, 4c, 512]
                nc.sync.dma_start(
                    se_b[:].rearrange("p (c h) -> p c h", c=4),
                    senc_d[a, sb0 : sb0 + 512, :].rearrange("(c p) h -> p c h", p=P),
                )
                se3 = se_b[:].rearrange("p (c h) -> p c h", c=4)
                if resident_values:
                    va3 = vres5[:, sc * 4 : (sc + 1) * 4, a].rearrange(
                        "p c k d -> p c (k d)"
                    )
                else:
                    va_b = chunkp.tile([P, 4 * H], f16, tag="values_all")
                    nc.sync.dma_start(
                        va_b[:].rearrange("p (c h) -> p c h", c=4),
                        vals_d[a, sb0 : sb0 + 512, :].rearrange(
                            "(c p) h -> p c h", p=P
                        ),
                    )
                    va3 = va_b[:].rearrange("p (c h) -> p c h", c=4)
                # cin LN stats per (b-row): [128, 4c] from phase-A accumulators
                sx = work.tile([P, 4], f32, tag="sx_cin")
                sx2 = work.tile([P, 4], f32, tag="sx2_cin")
                a0 = a * NCH + sc * 4
                nc.vector.tensor_tensor(
                    out=sx[:], in0=sxA[:, a0 : a0 + 4], in1=vsum[:, a0 : a0 + 4],
                    op=FP.add,
                )
                nc.vector.tensor_tensor(
                    out=sx2[:], in0=sesq[:, a0 : a0 + 4], in1=vasq[:, a0 : a0 + 4],
                    op=FP.add,
                )
                mean = work.tile([P, 4], f32, tag="mean_cin")
                var = work.tile([P, 4], f32, tag="var_cin")
                msq = work.tile([P, 4], f32, tag="msq_cin")
                rstd = work.tile([P, 4], f32, tag="rstd_cin")
                nc.vector.tensor_scalar_mul(mean[:], sx[:], inv_2h)
                nc.vector.tensor_scalar_mul(var[:], sx2[:], inv_2h)
                nc.vector.tensor_tensor(out=msq[:], in0=mean[:], in1=mean[:], op=FP.mult)
                nc.vector.tensor_tensor(out=var[:], in0=var[:], in1=msq[:], op=FP.subtract)
                nc.scalar.activation(msq[:], var[:], AF.Ln, bias=eps_t[:])
                nc.scalar.activation(rstd[:], msq[:], AF.Exp, scale=-0.5)

                # normalize + XBAR transpose -> cinT_all [128f, (cc4, ft8, 128b)]
                cinT = chunkp.tile([P, 4 * 8 * P], f16, tag="keys_all")
                for cc in range(4):
                    nrm = work.tile([P, 2 * H], f16, tag="nrm")
                    nc.vector.tensor_scalar(
                        out=nrm[:, :H],
                        in0=se3[:, cc, :],
                        scalar1=mean[:, cc : cc + 1],
                        scalar2=rstd[:, cc : cc + 1],
                        op0=FP.subtract,
                        op1=FP.mult,
                    )
                    nc.vector.tensor_scalar(
                        out=nrm[:, H:],
                        in0=va3[:, cc, :],
                        scalar1=mean[:, cc : cc + 1],
                        scalar2=rstd[:, cc : cc + 1],
                        op0=FP.subtract,
                        op1=FP.mult,
                    )
                    nc.sync.dma_start_transpose(
                        cinT[:, cc * 1024 : (cc + 1) * 1024].rearrange(
                            "p (t b) -> p t b", t=8
                        ),
                        nrm[:],
                    )

                # mm1: h1_T [128h x 4ht, 512b] = Wc1.T @ cinT
                h1T = chunkp.tile([P, 4 * 512], f16, tag="vals_all")
                h1T3 = h1T[:].rearrange("p (ht b) -> p ht b", ht=4)
                for ht in range(4):
                    ps_h1 = ps.tile([P, 512], f32, tag="mm")
                    for ft in range(8):
                        rhs = bass.AP(
                            cinT.tensor, cinT[:].offset + ft * P,
                            [list(cinT[:].ap[0]), [1024, 4], [1, P]],
                        )
                        nc.tensor.matmul(
                            ps_h1[:],
                            wc13[:, ft, ht * P : (ht + 1) * P],
                            rhs,
                            start=(ft == 0),
                            stop=(ft == 7),
                        )
                    nc.scalar.activation(
                        h1T3[:, ht, :], ps_h1[:], AF.Prelu,
                        bias=bias_c1[:, a * 4 + ht : a * 4 + ht + 1],
                        alpha=LRELU_SLOPE,
                    )
                # mm2: allqs_T [16, 512b]
                ps_q = ps.tile([ADIM, 512], f32, tag="mm")
                for ht in range(4):
                    nc.tensor.matmul(
                        ps_q[:],
                        wc23[:, ht, :],
                        h1T3[:, ht, :],
                        start=(ht == 0),
                        stop=(ht == 3),
                    )
                qs = work.tile([ADIM, 512], f32, tag="qs")
                nc.scalar.activation(
                    qs[:], ps_q[:], AF.Identity, bias=bias_c2[:, a : a + 1]
                )
                nc.sync.dma_start(allqs_d[a, :, sb0 : sb0 + 512], qs[:])

        # ================= PHASE A =================
        for c in range(NCH):
            b0 = c * P
            # batched input load: stac[:, a, 0:128]=states, [:, a, 128:144]=actions
            stac = io.tile([P, A * SA], f32, tag="stac")
            stac3 = stac[:].rearrange("p (a s) -> p a s", a=A)
            nc.sync.dma_start(
                stac3[:, :, :S], st_d[:, b0 : b0 + P, :].rearrange("a b s -> b a s")
            )
            nc.sync.dma_start(
                stac3[:, :, S:], ac_d[:, b0 : b0 + P, :].rearrange("a b s -> b a s")
            )
            st_t = [stac3[:, a, :S] for a in range(A)]
            ac_t = [stac3[:, a, S:] for a in range(A)]
            # one-pass grouped stats via bn_stats: 9 groups of 16 per agent
            agg_st = chunkp.tile([P, 2 * A], f32, tag="agg_st")  # (a, [mean,var])
            agg_sa = chunkp.tile([P, 2 * A], f32, tag="agg_sa")
            for a in range(A):
                bn6 = work.tile([P, 2 * 6], f32, tag="bn6")
                nc.vector.bn_stats(out=bn6[:, :6], in_=stac3[:, a, :S])
                nc.vector.bn_stats(out=bn6[:, 6:], in_=stac3[:, a, :])
                nc.vector.bn_aggr(out=agg_st[:, 2 * a : 2 * a + 2], in_=bn6[:, :6])
                nc.vector.bn_aggr(out=agg_sa[:, 2 * a : 2 * a + 2], in_=bn6[:, 6:])

            def ln_scalars(agg, tag):
                # rstd = exp(-0.5*ln(var+eps)) from strided var columns
                rstd = chunkp.tile([P, A], f32, tag=f"rstd_{tag}")
                lnv = chunkp.tile([P, A], f32, tag=f"lnv_{tag}")
                var_ap = bass.AP(agg.tensor, agg[:].offset + 1, [list(agg[:].ap[0]), [2, A]])
                nc.scalar.activation(lnv[:], var_ap, AF.Ln, bias=eps_t[:])
                nc.scalar.activation(rstd[:], lnv[:], AF.Exp, scale=-0.5)
                return rstd

            rstd_st = ln_scalars(agg_st, "st")
            rstd_sa = ln_scalars(agg_sa, "sa")
            mean_st = [agg_st[:, 2 * a : 2 * a + 1] for a in range(A)]
            mean_sa = [agg_sa[:, 2 * a : 2 * a + 1] for a in range(A)]

            # per-chunk shared attention inputs
            keys_all = chunkp.tile([P, A * H], f16, tag="keys_all")
            vals_all = chunkp.tile([P, A * H], f16, tag="vals_all")
            sel_all = chunkp.tile([P, A * H], f16, tag="sel_all")

            # normalize all agents into chunk-wide tiles, then XBAR-transpose
            stn_all = chunkp.tile([P, A * S], f16, tag="stn_all")
            san_all = chunkp.tile([P, A * S], f16, tag="san_all")
            sanTa_all = chunkp.tile([ADIM, A * P], f16, tag="sanTa_all")
            for a in range(A):
                st, ac = st_t[a], ac_t[a]
                nc.vector.tensor_scalar(
                    out=stn_all[:, a * S : (a + 1) * S],
                    in0=st,
                    scalar1=mean_st[a],
                    scalar2=rstd_st[:, a : a + 1],
                    op0=FP.subtract,
                    op1=FP.mult,
                )
                nc.vector.tensor_scalar(
                    out=san_all[:, a * S : (a + 1) * S],
                    in0=st,
                    scalar1=mean_sa[a],
                    scalar2=rstd_sa[:, a : a + 1],
                    op0=FP.subtract,
                    op1=FP.mult,
                )
                sanac = work.tile([P, ADIM], f16, tag="sanac")
                nc.vector.tensor_scalar(
                    out=sanac[:],
                    in0=ac,
                    scalar1=mean_sa[a],
                    scalar2=rstd_sa[:, a : a + 1],
                    op0=FP.subtract,
                    op1=FP.mult,
                )
                ps_ta = ps.tile([ADIM, P], f16, tag="tra")
                nc.tensor.transpose(ps_ta[:], sanac[:], ident[:])
                nc.vector.tensor_scalar_mul(
                    sanTa_all[:, a * P : (a + 1) * P], ps_ta[:], 1.0
                )
            stnT_all = chunkp.tile([P, A * P], f16, tag="stnT_all")
            nc.sync.dma_start_transpose(
                stnT_all[:].rearrange("p (t b) -> p t b", t=A), stn_all[:]
            )
            sanT_all = chunkp.tile([P, A * P], f16, tag="sanT_all")
            nc.sync.dma_start_transpose(
                sanT_all[:].rearrange("p (t b) -> p t b", t=A), san_all[:]
            )

            for a in range(A):
                stnT = stnT_all[:, a * P : (a + 1) * P]
                sanT = sanT_all[:, a * P : (a + 1) * P]
                sanTa = sanTa_all[:, a * P : (a + 1) * P]
                # s_enc = lrelu(stn @ Ws_s[a]) : [128b, 512]
                ps_se = ps.tile([P, H], f32, tag="mm")
                nc.tensor.matmul(
                    ps_se[:], stnT, w_ss[:, a * H : (a + 1) * H], start=True, stop=True
                )
                senc = work.tile([P, H], f16, tag="senc")
                if nonzero_bias:
                    tmp = work.tile([P, H], f32, tag="senc_tmp")
                    nc.vector.tensor_tensor(
                        out=tmp[:], in0=ps_se[:], in1=bias_ss[:, a * H : (a + 1) * H], op=FP.add
                    )
                    nc.scalar.activation(
                        senc[:], tmp[:], AF.Prelu, alpha=LRELU_SLOPE,
                        accum_out=sxA[:, a * NCH + c : a * NCH + c + 1],
                    )
                else:
                    nc.scalar.activation(
                        senc[:], ps_se[:], AF.Prelu, alpha=LRELU_SLOPE,
                        accum_out=sxA[:, a * NCH + c : a * NCH + c + 1],
                    )
                nc.gpsimd.dma_start(senc_d[a, b0 : b0 + P, :], senc[:])
                sqv = work.tile([P, H], f32, tag="sqv")
                nc.scalar.activation(
                    sqv[:], senc[:], AF.Square,
                    accum_out=sesq[:, a * NCH + c : a * NCH + c + 1],
                )

                # sa_enc = lrelu(san @ Ws_sa[a]) : [128b, 512]
                ps_sa = ps.tile([P, H], f32, tag="mm")
                nc.tensor.matmul(
                    ps_sa[:], sanT, w_ssa[:, a * H : (a + 1) * H], start=True, stop=False
                )
                nc.tensor.matmul(
                    ps_sa[:], sanTa, w_ssa_a[:, a * H : (a + 1) * H], start=False, stop=True
                )
                saenc = work.tile([P, H], f16, tag="saenc")
                if nonzero_bias:
                    tmp2 = work.tile([P, H], f32, tag="saenc_tmp")
                    nc.vector.tensor_tensor(
                        out=tmp2[:], in0=ps_sa[:], in1=bias_ssa[:, a * H : (a + 1) * H], op=FP.add
                    )
                    nc.scalar.activation(saenc[:], tmp2[:], AF.Prelu, alpha=LRELU_SLOPE)
                else:
                    nc.scalar.activation(saenc[:], ps_sa[:], AF.Prelu, alpha=LRELU_SLOPE)

                # transpose sa_enc -> [512h -> 4 tiles, 128b] via XBAR
                saencT = work.tile([P, 4 * P], f16, tag="saencT")  # [128h x 4ht, 128b]
                nc.sync.dma_start_transpose(
                    saencT[:].rearrange("p (t b) -> p t b", t=4), saenc[:]
                )
                saencT3 = saencT[:].rearrange("p (t b) -> p t b", t=4)

                # keys/vals/sel: [128b, 512(head,d)]
                for (wt, dst, act, kad) in (
                    (w_k, keys_all, False, True),
                    (w_v, vals_all, False, False),
                    (w_sel, sel_all, True, True),
                ):
                    ps_kv = ps.tile([P, H], f32, tag="mm")
                    wt3 = wt[:].rearrange("p (t n) -> p t n", t=4)
                    for t in range(4):
                        nc.tensor.matmul(
                            ps_kv[:],
                            saencT3[:, t, :],
                            wt3[:, t, :],
                            start=(t == 0),
                            stop=(t == 3),
                        )
                    if kad:
                        # layout (k, a, d): dst[:, k, a, d] strided per agent
                        dslice = bass.AP(
                            dst.tensor, dst[:].offset + a * D,
                            [list(dst[:].ap[0]), [A * D, HEADS], [1, D]],
                        )
                    else:
                        dslice = dst[:, a * H : (a + 1) * H]
                    if act:
                        if nonzero_bias:
                            tmp3 = work.tile([P, H], f32, tag="sel_tmp")
                            nc.vector.tensor_tensor(
                                out=tmp3[:], in0=ps_kv[:], in1=bias_sel[:], op=FP.add
                            )
                            nc.scalar.activation(dslice, tmp3[:], AF.Prelu, alpha=LRELU_SLOPE)
                        else:
                            nc.scalar.activation(dslice, ps_kv[:], AF.Prelu, alpha=LRELU_SLOPE)
                    else:
                        nc.scalar.copy(dslice, ps_kv[:])

            # ---- attention middle (per chunk): PE-batched logits ----
            # Per head: XBAR-transpose sel/keys slices to [d, (a), b]; batched
            # logits per 16-batch group g: stationary selT[d, (i, bm)], moving
            # keysT[d, (bm', j)]; out PSUM [(i,bm), (bm', j)]; bm'==bm useful.
            c2 = c % 2
            for k in range(HEADS):
                keysT = attn.tile([P, A * P], f16, tag="keysT")
                nc.sync.dma_start_transpose(
                    keysT[:].rearrange("p (t b) -> p t b", t=A),
                    keys_all[:, k * A * D : (k + 1) * A * D],
                )
                selT = attn.tile([P, A * P], f16, tag="selT")
                nc.sync.dma_start_transpose(
                    selT[:].rearrange("p (t b) -> p t b", t=A),
                    sel_all[:, k * A * D : (k + 1) * A * D],
                )
                # rearrange sel to selG[d, (g, i, bm)] so each group's
                # stationary is one contiguous 128-column block
                selG = attn.tile([P, A * P], f16, tag="selG")
                selg_eng = nc.vector if "gpselg" in _SKIP else nc.gpsimd
                for g in range(8):
                    selg_eng.tensor_tensor(
                        out=bass.AP(selG.tensor, selG[:].offset + g * P,
                                    [list(selG[:].ap[0]), [16, A], [1, 16]]),
                        in0=bass.AP(selT.tensor, selT[:].offset + g * 16,
                                    [list(selT[:].ap[0]), [P, A], [1, 16]]),
                        in1=bass.AP(zero16.tensor, zero16[:].offset,
                                    [list(zero16[:].ap[0]), [0, A], [0, 16]]),
                        op=FP.add,
                    )
                e_stage = scr.tile([P, 8 * P], f16, tag="e_stage")
                for g in range(8):
                    ps_l = psl.tile([P, P], f32, tag="lg")
                    lhs = selG[:, g * P : (g + 1) * P]
                    rhs = bass.AP(
                        keysT.tensor,
                        keysT[:].offset + g * 16,
                        [list(keysT[:].ap[0]), [1, 16], [P, A]],
                    )
                    nc.tensor.matmul(ps_l[:], lhs, rhs, start=True, stop=True)
                    nc.scalar.activation(
                        e_stage[:, g * P : (g + 1) * P],
                        ps_l[:],
                        AF.Exp,
                        scale=INV_SQRT_D,
                    )
                nc.sync.dma_start(ebig_d[c2, k, :, :], e_stage[:])
            e_st = scr.tile([P, HEADS * A * A], f16, tag="e_st")
            for g in range(8):
                # src (bm, (k,i) merged, j): addr = k*131072 + i*16384 + bm*1032 + g*128 + j
                src_ap = bass.AP(
                    ebig_d,
                    c2 * (HEADS * P * 1024) + g * P,
                    [[1024 + 8, 16], [16 * 1024, HEADS * A], [1, A]],
                )
                dbase = e_st[g * 16 : (g + 1) * 16, :]
                dst = bass.AP(
                    dbase.tensor, dbase.offset,
                    [list(dbase.ap[0]), [A, HEADS * A], [1, A]],
                )
                nc.gpsimd.dma_start(dst, src_ap)
            # zero self-attention entries e[k,i,i]
            diag = bass.AP(
                e_st.tensor,
                e_st[:].offset,
                [list(e_st[:].ap[0]), [A * A, HEADS], [A + 1, A]],
            )
            nc.vector.memset(diag, 0.0)
            # Z and 1/Z
            z_t = scr.tile([P, HEADS * A], f32, tag="z_t")  # [(k,i)]
            nc.vector.tensor_reduce(
                out=z_t[:],
                in_=e_st[:].rearrange("p (ki j) -> p ki j", j=A),
                axis=AX.X,
                op=FP.add,
            )
            rz = scr.tile([P, HEADS * A], f32, tag="rz")
            nc.vector.reciprocal(rz[:], z_t[:])
            # wexp[(k,i,j,s=2)] = e * rz (softmax weight, dsub-expanded x2)
            wexp = scr.tile([P, HEADS * A * A * 2], f16, tag="wexp")
            nc.vector.tensor_tensor(
                out=bass.AP(wexp.tensor, wexp[:].offset,
                            [list(wexp[:].ap[0]), [16, 32], [2, A], [1, 2]]),
                in0=bass.AP(e_st.tensor, e_st[:].offset,
                            [list(e_st[:].ap[0]), [8, 32], [1, A], [0, 2]]),
                in1=bass.AP(rz.tensor, rz[:].offset,
                            [list(rz[:].ap[0]), [1, 32], [0, A], [0, 2]]),
                op=FP.mult,
            )

            # qvals: values[i,(k,d)] = sum_j wexp[k,i,j] * vals[j,(k,d)]
            if resident_values:
                values4 = vres5[:, c]
            else:
                values_all = chunkp.tile([P, A * H], f16, tag="values_all")
                values4 = values_all[:].rearrange(
                    "p (i k d) -> p i k d", i=A, k=HEADS
                )
            scaled = scr.tile([P, A * A * D], f16, tag="scaled")  # (j,i,d) per k
            for k in range(HEADS):
                for i in range(A):
                    # scaled[:, j, i, d] = vals[:, j, k, d] * wexp[:, k, i, j, (s)]
                    nc.vector.tensor_tensor(
                        out=bass.AP(scaled.tensor, scaled[:].offset + i * D,
                                    [list(scaled[:].ap[0]), [A * D, A], [2, 64], [1, 2]]),
                        in0=bass.AP(vals_all.tensor, vals_all[:].offset + k * D,
                                    [list(vals_all[:].ap[0]), [H, A], [2, 64], [1, 2]]),
                        in1=bass.AP(wexp.tensor, wexp[:].offset + k * 128 + i * 16,
                                    [list(wexp[:].ap[0]), [2, A], [0, 64], [1, 2]]),
                        op=FP.mult,
                    )
                # tree-add over j, in place in `scaled`
                nc.vector.tensor_tensor(
                    out=scaled[:, : 4 * A * D], in0=scaled[:, : 4 * A * D],
                    in1=scaled[:, 4 * A * D :], op=FP.add,
                )
                nc.vector.tensor_tensor(
                    out=scaled[:, : 2 * A * D], in0=scaled[:, : 2 * A * D],
                    in1=scaled[:, 2 * A * D : 4 * A * D], op=FP.add,
                )
                nc.vector.tensor_tensor(
                    out=values4[:, :, k, :],
                    in0=scaled[:, : A * D].rearrange("p (i d) -> p i d", i=A),
                    in1=scaled[:, A * D : 2 * A * D].rearrange("p (i d) -> p i d", i=A),
                    op=FP.add,
                )

            # values stats for phase-B LN (sum + sum of squares per agent)
            nc.vector.tensor_reduce(
                out=bass.AP(vsum.tensor, vsum[:].offset + c,
                            [list(vsum[:].ap[0]), [NCH, A]]),
                in_=values_all[:].rearrange("p (i h) -> p i h", i=A),
                axis=AX.X,
                op=FP.add,
            )
            for i in range(A):
                sqv2 = work.tile([P, H], f32, tag="sqv")
                nc.scalar.activation(
                    sqv2[:], values_all[:, i * H : (i + 1) * H], AF.Square,
                    accum_out=vasq[:, i * NCH + c : i * NCH + c + 1],
                )

            nc.sync.dma_start(
                vals_d[:, b0 : b0 + P, :].rearrange("a b h -> b a h"),
                values_all[:].rearrange("p (a h) -> p a h", a=A),
            )


        for _sc in range(NSC):
            emit_phase_b(_sc)

    nc.compile()
    return nc


_CACHE = {}


def kernel(**inputs):
    states = np.asarray(inputs["states"], np.float32)
    actions = np.asarray(inputs["actions"], np.float32)
    Ws_s = np.asarray(inputs["Ws_s"], np.float32)
    bs_s = np.asarray(inputs["bs_s"], np.float32)
    Ws_sa = np.asarray(inputs["Ws_sa"], np.float32)
    bs_sa = np.asarray(inputs["bs_sa"], np.float32)
    Wk = np.asarray(inputs["Wk"], np.float32)
    Wv = np.asarray(inputs["Wv"], np.float32)
    Wsel = np.asarray(inputs["Wsel"], np.float32)
    bsel = np.asarray(inputs["bsel"], np.float32)
    Wc1 = np.asarray(inputs["Wc1"], np.float32)
    bc1 = np.asarray(inputs["bc1"], np.float32)
    Wc2 = np.asarray(inputs["Wc2"], np.float32)
    bc2 = np.asarray(inputs["bc2"], np.float32)

    nonzero_bias = bool(
        np.any(bs_s) or np.any(bs_sa) or np.any(bsel)
    )
    if nonzero_bias not in _CACHE:
        _CACHE[nonzero_bias] = build(nonzero_bias)
    nc = _CACHE[nonzero_bias]

    # host-side weight prep (shared across cores)
    # Wk/Wv/Wsel [HEADS, H, D] -> [H, (head, d)] fp16
    wk_f = np.ascontiguousarray(Wk.transpose(1, 0, 2).reshape(H, H)).astype(np.float16)
    wv_f = np.ascontiguousarray(Wv.transpose(1, 0, 2).reshape(H, H)).astype(np.float16)
    wsel_f = np.ascontiguousarray(Wsel.transpose(1, 0, 2).reshape(H, H)).astype(np.float16)
    shared = {
        "ws_s": Ws_s.astype(np.float16),
        "ws_sa": Ws_sa.astype(np.float16),
        "wk": wk_f,
        "wv": wv_f,
        "wsel": wsel_f,
        "wc1": Wc1.astype(np.float16),
        "wc2": Wc2.astype(np.float16),
        "bs_s": bs_s.astype(np.float16),
        "bs_sa": bs_sa.astype(np.float16),
        "bsel": bsel.astype(np.float16),
        "bc1": bc1,
        "bc2": bc2,
    }
    in_maps = []
    for core in range(NCORES):
        sl = slice(core * BL, (core + 1) * BL)
        m = dict(shared)
        m["states"] = np.ascontiguousarray(states[:, sl, :])
        m["actions"] = np.ascontiguousarray(actions[:, sl, :])
        in_maps.append(m)

    res = run_bass_kernel_spmd(nc, in_maps, core_ids=list(range(NCORES)))
    # gather: q[a, b] = allqs[a, idx[a, b], b]
    idx = np.argmax(actions, axis=-1)  # [A, B]
    q = np.empty((A, B, 1), np.float32)
    for core in range(NCORES):
        allqs = res.results[core]["allqs"]  # [A, ADIM, BL]
        sl = slice(core * BL, (core + 1) * BL)
        ii = idx[:, sl]
        q[:, sl, 0] = np.take_along_axis(
            allqs, ii[:, None, :], axis=1
        )[:, 0, :]
    return q


def _install_ntff_hook():
    """The agent image's antenv lacks axon_hooks; synthesize it so
    run_bass_kernel_spmd(trace=True) can capture NTFF profiles."""
    import types
    import antenv

    if "antenv.axon_hooks" in sys.modules:
        return
    mod = types.ModuleType("antenv.axon_hooks")
    mod._hook = None

    def set_axon_ntff_profile_hook(h):
        mod._hook = h

    def get_axon_ntff_profile_hook():
        return mod._hook

    mod.set_axon_ntff_profile_hook = set_axon_ntff_profile_hook
    mod.get_axon_ntff_profile_hook = get_axon_ntff_profile_hook
    sys.modules["antenv.axon_hooks"] = mod
    antenv.axon_hooks = mod
    sys.path.insert(0, "/root/.axon_site")
    from trn_agent_boot.trn_boot import _ntff_profile_via_ctypes

    hook = _ntff_profile_via_ctypes("/opt/axon/libaxon_pjrt.so")
    if hook is not None:
        set_axon_ntff_profile_hook(hook)


def profile_run(inputs):
    """Traced run returning HW exec time in ns (max across cores)."""
    import os
    os.environ["BASS_PERFETTO_PROFILE_ALL_CORES"] = "1"
    _install_ntff_hook()
    states = np.asarray(inputs["states"], np.float32)
    actions = np.asarray(inputs["actions"], np.float32)
    nonzero_bias = bool(
        np.any(inputs["bs_s"]) or np.any(inputs["bs_sa"]) or np.any(inputs["bsel"])
    )
    if nonzero_bias not in _CACHE:
        _CACHE[nonzero_bias] = build(nonzero_bias)
    nc = _CACHE[nonzero_bias]
    wk_f = np.ascontiguousarray(np.asarray(inputs["Wk"], np.float32).transpose(1, 0, 2).reshape(H, H)).astype(np.float16)
    wv_f = np.ascontiguousarray(np.asarray(inputs["Wv"], np.float32).transpose(1, 0, 2).reshape(H, H)).astype(np.float16)
    wsel_f = np.ascontiguousarray(np.asarray(inputs["Wsel"], np.float32).transpose(1, 0, 2).reshape(H, H)).astype(np.float16)
    shared = {
        "ws_s": np.asarray(inputs["Ws_s"], np.float32).astype(np.float16),
        "ws_sa": np.asarray(inputs["Ws_sa"], np.float32).astype(np.float16),
        "wk": wk_f, "wv": wv_f, "wsel": wsel_f,
        "wc1": np.asarray(inputs["Wc1"], np.float32).astype(np.float16),
        "wc2": np.asarray(inputs["Wc2"], np.float32).astype(np.float16),
        "bs_s": np.asarray(inputs["bs_s"], np.float16),
        "bs_sa": np.asarray(inputs["bs_sa"], np.float16),
        "bsel": np.asarray(inputs["bsel"], np.float16),
        "bc1": np.asarray(inputs["bc1"], np.float32),
        "bc2": np.asarray(inputs["bc2"], np.float32),
    }
    in_maps = []
    for core in range(NCORES):
        sl = slice(core * BL, (core + 1) * BL)
        m = dict(shared)
        m["states"] = np.ascontiguousarray(states[:, sl, :])
        m["actions"] = np.ascontiguousarray(actions[:, sl, :])
        in_maps.append(m)
    res = run_bass_kernel_spmd(
        nc, in_maps, core_ids=list(range(NCORES)), trace=True,
        tmpdir="/tmp/prof", trace_cores=[0],
    )
    print("mean exec:", res.mean_exec_time_ns, "max core:", res.max_exec_time_core_id)
    return res.exec_time_ns



# revision 26
# speedup vs baseline: 1.0993x; 1.0993x over previous
"""AttentionCritic Trainium2 kernel.

Problem (hardcoded): A=8 agents, B=8192 batch, S=128 state, ADIM=16 act,
H=512 hid, HEADS=4, D=128. 8 NeuronCores, batch-sharded (1024 batch/core).

Pipeline per core (b = 1024 local batch, chunks of 128):
  Phase A (chunk-outer, agent-inner):
    LN(states), LN([states|actions]) in fp32, normalized out in fp16;
    PE-transpose normalized inputs; fp16 matmuls for s_enc / sa_enc
    (batch-major out via activation-as-stationary); PE-transpose sa_enc;
    keys/vals/sel projections (all heads at once); attention middle on DVE
    (broadcast-AP mul + segmented reduce for logits, tensor_scalar +
    tree-add for weighted values); Prelu evictions on ACT. s_enc and
    attended values bounce through DRAM in fp16.
  Phase B (agent-outer, 512-batch superchunks):
    LN(cin) stats in batch-major, normalize, PE-transpose, critic matmuls
    (h1 feature-major so bc1 folds into the Prelu eviction), all_qs out.
  Host: shard/unshard, fp16 weight casts, argmax-gather of all_qs.
"""
import sys

sys.path.insert(0, "/opt/trn_rl_repo")
from contextlib import ExitStack

import numpy as np

import concourse.bass as bass
import concourse.bacc as bacc
import concourse.mybir as mybir
from concourse import tile
from concourse.bass_utils import run_bass_kernel_spmd
from concourse.masks import make_identity

A, B, S, ADIM = 8, 8192, 128, 16
H, HEADS = 512, 4
D = H // HEADS
EPS = 1e-5
NCORES = 8
BL = B // NCORES          # local batch per core = 1024
P = 128                   # partition size
NCH = BL // P             # chunks per core = 8
NSC = BL // 512           # 512-batch superchunks = 2
SA = S + ADIM             # 144

f32 = mybir.dt.float32
f16 = mybir.dt.float16
FP = mybir.AluOpType
AF = mybir.ActivationFunctionType
AX = mybir.AxisListType
LRELU_SLOPE = 0.01
INV_SQRT_D = 1.0 / float(np.sqrt(D))


def _bc(ap, axis, n):
    """Insert a broadcast (step 0, count n) free dim at position `axis`
    (0 = first free dim)."""
    dims = [list(d) for d in ap.ap]
    dims.insert(1 + axis, [0, n])
    return bass.AP(ap.tensor, ap.offset, dims)


def _bc_front(ap, n):
    """Prepend a broadcast dim (for DRAM->SBUF partition replication)."""
    dims = [[0, n]] + [list(d) for d in ap.ap]
    return bass.AP(ap.tensor, ap.offset, dims)


def _patch_act_tables():
    """Restrict bacc's activation-table choices to the one set that covers
    every func we use (exp, ln, square, parametric_relu, identity, copy) so
    no ACT_TABLE_LOAD churn happens mid-kernel."""
    from concourse import hw_specs
    import concourse.bacc as _bacc

    orig = hw_specs.get_activation_tables

    def only_combined(arch):
        t = orig(arch)
        if "natural_log_exp_and_others" not in t:
            return t
        # keep ordering (act_func_set_id indexes the real act_info.json) but
        # make every other set uncoverable so the chooser sticks to one table
        return {
            k: (v if k == "natural_log_exp_and_others" else set())
            for k, v in t.items()
        }

    only_combined.__wrapped__ = orig
    hw_specs.get_activation_tables = only_combined
    _bacc.get_activation_tables = only_combined


import os
_SKIP = set(os.environ.get("KSKIP", "").split(","))


def build(nonzero_bias):
    if "tables" not in _SKIP:
        _patch_act_tables()
    nc = bacc.Bacc("TRN2", target_bir_lowering=False, debug=False)

    # ---- DRAM I/O ----
    st_d = nc.dram_tensor("states", [A, BL, S], f32, kind="ExternalInput")
    ac_d = nc.dram_tensor("actions", [A, BL, ADIM], f32, kind="ExternalInput")
    ws_s_d = nc.dram_tensor("ws_s", [A, S, H], f16, kind="ExternalInput")
    ws_sa_d = nc.dram_tensor("ws_sa", [A, SA, H], f16, kind="ExternalInput")
    wk_d = nc.dram_tensor("wk", [H, H], f16, kind="ExternalInput")   # [h, (head,d)]
    wv_d = nc.dram_tensor("wv", [H, H], f16, kind="ExternalInput")
    wsel_d = nc.dram_tensor("wsel", [H, H], f16, kind="ExternalInput")
    wc1_d = nc.dram_tensor("wc1", [A, 2 * H, H], f16, kind="ExternalInput")
    wc2_d = nc.dram_tensor("wc2", [A, H, ADIM], f16, kind="ExternalInput")
    # biases (replicated across partitions host-side where needed)
    bs_s_d = nc.dram_tensor("bs_s", [A, H], f16, kind="ExternalInput")
    bs_sa_d = nc.dram_tensor("bs_sa", [A, H], f16, kind="ExternalInput")
    bsel_d = nc.dram_tensor("bsel", [H], f16, kind="ExternalInput")
    bc1_d = nc.dram_tensor("bc1", [A, H], f32, kind="ExternalInput")
    bc2_d = nc.dram_tensor("bc2", [A, ADIM], f32, kind="ExternalInput")

    allqs_d = nc.dram_tensor("allqs", [A, ADIM, BL], f32, kind="ExternalOutput")

    # DRAM scratch (internal)
    senc_d = nc.dram_tensor("senc_scr", [A, BL, H], f16)
    # logits bounce buffer for the diagonal gather (double-buffered by chunk)
    ebig_d = nc.dram_tensor("ebig_scr", [2, HEADS, P, 8 * 16 * A], f16)
    resident_values = False
    vals_d = None if resident_values else nc.dram_tensor("vals_scr", [A, BL, H], f16)

    with tile.TileContext(nc) as tc, ExitStack() as ctx:
        const = ctx.enter_context(tc.tile_pool(name="const", bufs=1))
        wpool = ctx.enter_context(tc.tile_pool(name="weights", bufs=1))
        io = ctx.enter_context(tc.tile_pool(name="io", bufs=2))
        scr = ctx.enter_context(tc.tile_pool(name="scr", bufs=1))
        work = ctx.enter_context(tc.tile_pool(name="work", bufs=2))
        chunkp = ctx.enter_context(tc.tile_pool(name="chunk", bufs=2))
        attn = ctx.enter_context(tc.tile_pool(name="attn", bufs=2))
        ps = ctx.enter_context(tc.tile_pool(name="ps", bufs=2, space="PSUM"))
        psl = ctx.enter_context(tc.tile_pool(name="psl", bufs=2, space="PSUM"))

        ident = const.tile([P, P], f16)
        make_identity(nc, ident[:])
        eps_t = const.tile([P, 1], f32)
        nc.vector.memset(eps_t[:], EPS)
        zero16 = const.tile([P, 1], f16)
        nc.vector.memset(zero16[:], 0.0)

        # ---- resident weights ----
        # w_ss: [S=128 partitions, (a, h)] ; per-agent rhs slice [128, 512]
        w_ss = wpool.tile([P, A * H], f16, tag="w_ss")
        nc.sync.dma_start(w_ss[:].rearrange("p (a h) -> p a h", a=A), ws_s_d[:].rearrange("a s h -> s a h"))
        # w_ssa split: state rows [128, (a,h)], action rows [16, (a,h)]
        w_ssa = wpool.tile([P, A * H], f16, tag="w_ssa")
        nc.sync.dma_start(w_ssa[:].rearrange("p (a h) -> p a h", a=A), ws_sa_d[:, :S, :].rearrange("a s h -> s a h"))
        w_ssa_a = wpool.tile([ADIM, A * H], f16, tag="w_ssa_a")
        nc.sync.dma_start(w_ssa_a[:].rearrange("p (a h) -> p a h", a=A), ws_sa_d[:, S:, :].rearrange("a s h -> s a h"))
        # w_k/v/sel: [h=512 -> 4 tiles of 128, (head,d)=512]
        w_k = wpool.tile([P, 4 * H], f16, tag="w_k")
        nc.sync.dma_start(w_k[:].rearrange("p (t n) -> p t n", t=4), wk_d[:].rearrange("(t p) n -> p t n", p=P))
        w_v = wpool.tile([P, 4 * H], f16, tag="w_v")
        nc.sync.dma_start(w_v[:].rearrange("p (t n) -> p t n", t=4), wv_d[:].rearrange("(t p) n -> p t n", p=P))
        w_sel = wpool.tile([P, 4 * H], f16, tag="w_sel")
        nc.sync.dma_start(w_sel[:].rearrange("p (t n) -> p t n", t=4), wsel_d[:].rearrange("(t p) n -> p t n", p=P))

        if nonzero_bias:
            bias_ss = wpool.tile([P, A * H], f16, tag="b_ss")
            nc.sync.dma_start(bias_ss[:], _bc_front(bs_s_d[:].rearrange("a h -> (a h)"), P))
            bias_ssa = wpool.tile([P, A * H], f16, tag="b_ssa")
            nc.sync.dma_start(bias_ssa[:], _bc_front(bs_sa_d[:].rearrange("a h -> (a h)"), P))
            bias_sel = wpool.tile([P, H], f16, tag="b_sel")
            nc.sync.dma_start(bias_sel[:], _bc_front(bsel_d[:], P))
        bias_c1 = wpool.tile([P, A * 4], f32, tag="b_c1")  # feature-major [128h x (a,ht)]
        nc.sync.dma_start(
            bias_c1[:].rearrange("p (a ht) -> p a ht", a=A),
            bc1_d[:].rearrange("a (ht p) -> p a ht", p=P),
        )
        bias_c2 = wpool.tile([ADIM, A], f32, tag="b_c2")
        nc.sync.dma_start(bias_c2[:], bc2_d[:].rearrange("a o -> o a"))

        inv_s = 1.0 / S
        inv_sa = 1.0 / SA
        inv_2h = 1.0 / (2 * H)
        sxA = wpool.tile([P, A * NCH], f32, tag="sxA")  # sum_h s_enc per (a, chunk)
        sesq = wpool.tile([P, A * NCH], f32, tag="sesq")   # sum_h s_enc^2
        vsum = wpool.tile([P, A * NCH], f32, tag="vsum")   # sum_h values
        vasq = wpool.tile([P, A * NCH], f32, tag="vasq")   # sum_h values^2

        def emit_phase_b(sc):
            for a in range(A):
                wc1 = scr.tile([P, 8 * H], f16, tag="scaled")  # [128f x 8ft, 512h]
                nc.sync.dma_start(
                wc1[:].rearrange("p (ft h) -> p ft h", ft=8),
                wc1_d[a, :, :].rearrange("(ft p) h -> p ft h", p=P),
                )
                wc2 = scr.tile([P, 4 * ADIM], f16, tag="t2")  # [128h x 4ht, 16]
                nc.sync.dma_start(
                wc2[:].rearrange("p (ht o) -> p ht o", ht=4),
                wc2_d[a, :, :].rearrange("(ht p) o -> p ht o", p=P),
                )
                wc13 = wc1[:].rearrange("p (ft h) -> p ft h", ft=8)
                wc23 = wc2[:].rearrange("p (ht o) -> p ht o", ht=4)
                sb0 = sc * 512
                se_b = chunkp.tile([P, 4 * H], f16, tag="sel_all")  # [128, 4c, 512]
                nc.sync.dma_start(
                    se_b[:].rearrange("p (c h) -> p c h", c=4),
                    senc_d[a, sb0 : sb0 + 512, :].rearrange("(c p) h -> p c h", p=P),
                )
                se3 = se_b[:].rearrange("p (c h) -> p c h", c=4)
                if resident_values:
                    va3 = vres5[:, sc * 4 : (sc + 1) * 4, a].rearrange(
                        "p c k d -> p c (k d)"
                    )
                else:
                    va_b = chunkp.tile([P, 4 * H], f16, tag="values_all")
                    nc.sync.dma_start(
                        va_b[:].rearrange("p (c h) -> p c h", c=4),
                        vals_d[a, sb0 : sb0 + 512, :].rearrange(
                            "(c p) h -> p c h", p=P
                        ),
                    )
                    va3 = va_b[:].rearrange("p (c h) -> p c h", c=4)
                # cin LN stats per (b-row): [128, 4c] from phase-A accumulators
                sx = work.tile([P, 4], f32, tag="sx_cin")
                sx2 = work.tile([P, 4], f32, tag="sx2_cin")
                a0 = a * NCH + sc * 4
                nc.vector.tensor_tensor(
                    out=sx[:], in0=sxA[:, a0 : a0 + 4], in1=vsum[:, a0 : a0 + 4],
                    op=FP.add,
                )
                nc.vector.tensor_tensor(
                    out=sx2[:], in0=sesq[:, a0 : a0 + 4], in1=vasq[:, a0 : a0 + 4],
                    op=FP.add,
                )
                mean = work.tile([P, 4], f32, tag="mean_cin")
                var = work.tile([P, 4], f32, tag="var_cin")
                msq = work.tile([P, 4], f32, tag="msq_cin")
                rstd = work.tile([P, 4], f32, tag="rstd_cin")
                nc.vector.tensor_scalar_mul(mean[:], sx[:], inv_2h)
                nc.vector.tensor_scalar_mul(var[:], sx2[:], inv_2h)
                nc.vector.tensor_tensor(out=msq[:], in0=mean[:], in1=mean[:], op=FP.mult)
                nc.vector.tensor_tensor(out=var[:], in0=var[:], in1=msq[:], op=FP.subtract)
                nc.scalar.activation(msq[:], var[:], AF.Ln, bias=eps_t[:])
                nc.scalar.activation(rstd[:], msq[:], AF.Exp, scale=-0.5)

                # normalize + XBAR transpose -> cinT_all [128f, (cc4, ft8, 128b)]
                cinT = chunkp.tile([P, 4 * 8 * P], f16, tag="keys_all")
                for cc in range(4):
                    nrm = work.tile([P, 2 * H], f16, tag="nrm")
                    nc.vector.tensor_scalar(
                        out=nrm[:, :H],
                        in0=se3[:, cc, :],
                        scalar1=mean[:, cc : cc + 1],
                        scalar2=rstd[:, cc : cc + 1],
                        op0=FP.subtract,
                        op1=FP.mult,
                    )
                    nc.vector.tensor_scalar(
                        out=nrm[:, H:],
                        in0=va3[:, cc, :],
                        scalar1=mean[:, cc : cc + 1],
                        scalar2=rstd[:, cc : cc + 1],
                        op0=FP.subtract,
                        op1=FP.mult,
                    )
                    nc.sync.dma_start_transpose(
                        cinT[:, cc * 1024 : (cc + 1) * 1024].rearrange(
                            "p (t b) -> p t b", t=8
                        ),
                        nrm[:],
                    )

                # mm1: h1_T [128h x 4ht, 512b] = Wc1.T @ cinT
                h1T = chunkp.tile([P, 4 * 512], f16, tag="vals_all")
                h1T3 = h1T[:].rearrange("p (ht b) -> p ht b", ht=4)
                for ht in range(4):
                    ps_h1 = ps.tile([P, 512], f32, tag="mm")
                    for ft in range(8):
                        rhs = bass.AP(
                            cinT.tensor, cinT[:].offset + ft * P,
                            [list(cinT[:].ap[0]), [1024, 4], [1, P]],
                        )
                        nc.tensor.matmul(
                            ps_h1[:],
                            wc13[:, ft, ht * P : (ht + 1) * P],
                            rhs,
                            start=(ft == 0),
                            stop=(ft == 7),
                        )
                    nc.scalar.activation(
                        h1T3[:, ht, :], ps_h1[:], AF.Prelu,
                        bias=bias_c1[:, a * 4 + ht : a * 4 + ht + 1],
                        alpha=LRELU_SLOPE,
                    )
                # mm2: allqs_T [16, 512b]
                ps_q = ps.tile([ADIM, 512], f32, tag="mm")
                for ht in range(4):
                    nc.tensor.matmul(
                        ps_q[:],
                        wc23[:, ht, :],
                        h1T3[:, ht, :],
                        start=(ht == 0),
                        stop=(ht == 3),
                    )
                qs = work.tile([ADIM, 512], f32, tag="qs")
                nc.scalar.activation(
                    qs[:], ps_q[:], AF.Identity, bias=bias_c2[:, a : a + 1]
                )
                nc.sync.dma_start(allqs_d[a, :, sb0 : sb0 + 512], qs[:])

        # ================= PHASE A =================
        for c in range(NCH):
            b0 = c * P
            # batched input load: stac[:, a, 0:128]=states, [:, a, 128:144]=actions
            stac = io.tile([P, A * SA], f32, tag="stac")
            stac3 = stac[:].rearrange("p (a s) -> p a s", a=A)
            nc.sync.dma_start(
                stac3[:, :, :S], st_d[:, b0 : b0 + P, :].rearrange("a b s -> b a s")
            )
            nc.sync.dma_start(
                stac3[:, :, S:], ac_d[:, b0 : b0 + P, :].rearrange("a b s -> b a s")
            )
            st_t = [stac3[:, a, :S] for a in range(A)]
            ac_t = [stac3[:, a, S:] for a in range(A)]
            # one-pass grouped stats via bn_stats: 9 groups of 16 per agent
            agg_st = chunkp.tile([P, 2 * A], f32, tag="agg_st")  # (a, [mean,var])
            agg_sa = chunkp.tile([P, 2 * A], f32, tag="agg_sa")
            for a in range(A):
                bn6 = work.tile([P, 2 * 6], f32, tag="bn6")
                nc.vector.bn_stats(out=bn6[:, :6], in_=stac3[:, a, :S])
                nc.vector.bn_stats(out=bn6[:, 6:], in_=stac3[:, a, :])
                nc.vector.bn_aggr(out=agg_st[:, 2 * a : 2 * a + 2], in_=bn6[:, :6])
                nc.vector.bn_aggr(out=agg_sa[:, 2 * a : 2 * a + 2], in_=bn6[:, 6:])

            def ln_scalars(agg, tag):
                # rstd = exp(-0.5*ln(var+eps)) from strided var columns
                rstd = chunkp.tile([P, A], f32, tag=f"rstd_{tag}")
                lnv = chunkp.tile([P, A], f32, tag=f"lnv_{tag}")
                var_ap = bass.AP(agg.tensor, agg[:].offset + 1, [list(agg[:].ap[0]), [2, A]])
                nc.scalar.activation(lnv[:], var_ap, AF.Ln, bias=eps_t[:])
                nc.scalar.activation(rstd[:], lnv[:], AF.Exp, scale=-0.5)
                return rstd

            rstd_st = ln_scalars(agg_st, "st")
            rstd_sa = ln_scalars(agg_sa, "sa")
            mean_st = [agg_st[:, 2 * a : 2 * a + 1] for a in range(A)]
            mean_sa = [agg_sa[:, 2 * a : 2 * a + 1] for a in range(A)]

            # per-chunk shared attention inputs
            keys_all = chunkp.tile([P, A * H], f16, tag="keys_all")
            vals_all = chunkp.tile([P, A * H], f16, tag="vals_all")
            sel_all = chunkp.tile([P, A * H], f16, tag="sel_all")

            # normalize all agents into chunk-wide tiles, then XBAR-transpose
            stn_all = chunkp.tile([P, A * S], f16, tag="stn_all")
            san_all = chunkp.tile([P, A * S], f16, tag="san_all")
            sanTa_all = chunkp.tile([ADIM, A * P], f16, tag="sanTa_all")
            for a in range(A):
                st, ac = st_t[a], ac_t[a]
                nc.vector.tensor_scalar(
                    out=stn_all[:, a * S : (a + 1) * S],
                    in0=st,
                    scalar1=mean_st[a],
                    scalar2=rstd_st[:, a : a + 1],
                    op0=FP.subtract,
                    op1=FP.mult,
                )
                nc.vector.tensor_scalar(
                    out=san_all[:, a * S : (a + 1) * S],
                    in0=st,
                    scalar1=mean_sa[a],
                    scalar2=rstd_sa[:, a : a + 1],
                    op0=FP.subtract,
                    op1=FP.mult,
                )
                sanac = work.tile([P, ADIM], f16, tag="sanac")
                nc.vector.tensor_scalar(
                    out=sanac[:],
                    in0=ac,
                    scalar1=mean_sa[a],
                    scalar2=rstd_sa[:, a : a + 1],
                    op0=FP.subtract,
                    op1=FP.mult,
                )
                ps_ta = ps.tile([ADIM, P], f16, tag="tra")
                nc.tensor.transpose(ps_ta[:], sanac[:], ident[:])
                nc.vector.tensor_scalar_mul(
                    sanTa_all[:, a * P : (a + 1) * P], ps_ta[:], 1.0
                )
            stnT_all = chunkp.tile([P, A * P], f16, tag="stnT_all")
            nc.sync.dma_start_transpose(
                stnT_all[:].rearrange("p (t b) -> p t b", t=A), stn_all[:]
            )
            sanT_all = chunkp.tile([P, A * P], f16, tag="sanT_all")
            nc.sync.dma_start_transpose(
                sanT_all[:].rearrange("p (t b) -> p t b", t=A), san_all[:]
            )

            for a in range(A):
                stnT = stnT_all[:, a * P : (a + 1) * P]
                sanT = sanT_all[:, a * P : (a + 1) * P]
                sanTa = sanTa_all[:, a * P : (a + 1) * P]
                # s_enc = lrelu(stn @ Ws_s[a]) : [128b, 512]
                ps_se = ps.tile([P, H], f32, tag="mm")
                nc.tensor.matmul(
                    ps_se[:], stnT, w_ss[:, a * H : (a + 1) * H], start=True, stop=True
                )
                senc = work.tile([P, H], f16, tag="senc")
                if nonzero_bias:
                    tmp = work.tile([P, H], f32, tag="senc_tmp")
                    nc.vector.tensor_tensor(
                        out=tmp[:], in0=ps_se[:], in1=bias_ss[:, a * H : (a + 1) * H], op=FP.add
                    )
                    nc.scalar.activation(
                        senc[:], tmp[:], AF.Prelu, alpha=LRELU_SLOPE,
                        accum_out=sxA[:, a * NCH + c : a * NCH + c + 1],
                    )
                else:
                    nc.scalar.activation(
                        senc[:], ps_se[:], AF.Prelu, alpha=LRELU_SLOPE,
                        accum_out=sxA[:, a * NCH + c : a * NCH + c + 1],
                    )
                nc.gpsimd.dma_start(senc_d[a, b0 : b0 + P, :], senc[:])
                sqv = work.tile([P, H], f32, tag="sqv")
                nc.scalar.activation(
                    sqv[:], senc[:], AF.Square,
                    accum_out=sesq[:, a * NCH + c : a * NCH + c + 1],
                )

                # sa_enc = lrelu(san @ Ws_sa[a]) : [128b, 512]
                ps_sa = ps.tile([P, H], f32, tag="mm")
                nc.tensor.matmul(
                    ps_sa[:], sanT, w_ssa[:, a * H : (a + 1) * H], start=True, stop=False
                )
                nc.tensor.matmul(
                    ps_sa[:], sanTa, w_ssa_a[:, a * H : (a + 1) * H], start=False, stop=True
                )
                saenc = work.tile([P, H], f16, tag="saenc")
                if nonzero_bias:
                    tmp2 = work.tile([P, H], f32, tag="saenc_tmp")
                    nc.vector.tensor_tensor(
                        out=tmp2[:], in0=ps_sa[:], in1=bias_ssa[:, a * H : (a + 1) * H], op=FP.add
                    )
                    nc.scalar.activation(saenc[:], tmp2[:], AF.Prelu, alpha=LRELU_SLOPE)
                else:
                    nc.scalar.activation(saenc[:], ps_sa[:], AF.Prelu, alpha=LRELU_SLOPE)

                # transpose sa_enc -> [512h, 128b]
                ps_saT = ps.tile([P, H], f16, tag="tr")
                for t in range(4):
                    nc.tensor.transpose(
                        ps_saT[:, t * P : (t + 1) * P],
                        saenc[:, t * P : (t + 1) * P],
                        ident[:],
                    )
                saencT = work.tile([P, 4 * P], f16, tag="saencT")  # [128h x 4ht, 128b]
                nc.scalar.copy(saencT[:], ps_saT[:])
                saencT3 = saencT[:].rearrange("p (t b) -> p t b", t=4)

                # keys/vals/sel: [128b, 512(head,d)]
                for (wt, dst, act, kad) in (
                    (w_k, keys_all, False, True),
                    (w_v, vals_all, False, False),
                    (w_sel, sel_all, True, True),
                ):
                    ps_kv = ps.tile([P, H], f32, tag="mm")
                    wt3 = wt[:].rearrange("p (t n) -> p t n", t=4)
                    for t in range(4):
                        nc.tensor.matmul(
                            ps_kv[:],
                            saencT3[:, t, :],
                            wt3[:, t, :],
                            start=(t == 0),
                            stop=(t == 3),
                        )
                    if kad:
                        # layout (k, a, d): dst[:, k, a, d] strided per agent
                        dslice = bass.AP(
                            dst.tensor, dst[:].offset + a * D,
                            [list(dst[:].ap[0]), [A * D, HEADS], [1, D]],
                        )
                    else:
                        dslice = dst[:, a * H : (a + 1) * H]
                    if act:
                        if nonzero_bias:
                            tmp3 = work.tile([P, H], f32, tag="sel_tmp")
                            nc.vector.tensor_tensor(
                                out=tmp3[:], in0=ps_kv[:], in1=bias_sel[:], op=FP.add
                            )
                            nc.scalar.activation(dslice, tmp3[:], AF.Prelu, alpha=LRELU_SLOPE)
                        else:
                            nc.scalar.activation(dslice, ps_kv[:], AF.Prelu, alpha=LRELU_SLOPE)
                    else:
                        nc.scalar.copy(dslice, ps_kv[:])

            # ---- attention middle (per chunk): PE-batched logits ----
            # Per head: XBAR-transpose sel/keys slices to [d, (a), b]; batched
            # logits per 16-batch group g: stationary selT[d, (i, bm)], moving
            # keysT[d, (bm', j)]; out PSUM [(i,bm), (bm', j)]; bm'==bm useful.
            c2 = c % 2
            for k in range(HEADS):
                keysT = attn.tile([P, A * P], f16, tag="keysT")
                nc.sync.dma_start_transpose(
                    keysT[:].rearrange("p (t b) -> p t b", t=A),
                    keys_all[:, k * A * D : (k + 1) * A * D],
                )
                selT = attn.tile([P, A * P], f16, tag="selT")
                nc.sync.dma_start_transpose(
                    selT[:].rearrange("p (t b) -> p t b", t=A),
                    sel_all[:, k * A * D : (k + 1) * A * D],
                )
                # rearrange sel to selG[d, (g, i, bm)] so each group's
                # stationary is one contiguous 128-column block
                selG = attn.tile([P, A * P], f16, tag="selG")
                selg_eng = nc.vector if "gpselg" in _SKIP else nc.gpsimd
                for g in range(8):
                    selg_eng.tensor_tensor(
                        out=bass.AP(selG.tensor, selG[:].offset + g * P,
                                    [list(selG[:].ap[0]), [16, A], [1, 16]]),
                        in0=bass.AP(selT.tensor, selT[:].offset + g * 16,
                                    [list(selT[:].ap[0]), [P, A], [1, 16]]),
                        in1=bass.AP(zero16.tensor, zero16[:].offset,
                                    [list(zero16[:].ap[0]), [0, A], [0, 16]]),
                        op=FP.add,
                    )
                e_stage = scr.tile([P, 8 * P], f16, tag="e_stage")
                for g in range(8):
                    ps_l = psl.tile([P, P], f32, tag="lg")
                    lhs = selG[:, g * P : (g + 1) * P]
                    rhs = bass.AP(
                        keysT.tensor,
                        keysT[:].offset + g * 16,
                        [list(keysT[:].ap[0]), [1, 16], [P, A]],
                    )
                    nc.tensor.matmul(ps_l[:], lhs, rhs, start=True, stop=True)
                    nc.scalar.activation(
                        e_stage[:, g * P : (g + 1) * P],
                        ps_l[:],
                        AF.Exp,
                        scale=INV_SQRT_D,
                    )
                nc.sync.dma_start(ebig_d[c2, k, :, :], e_stage[:])
            e_st = scr.tile([P, HEADS * A * A], f16, tag="e_st")
            for g in range(8):
                # src (bm, (k,i) merged, j): addr = k*131072 + i*16384 + bm*1032 + g*128 + j
                src_ap = bass.AP(
                    ebig_d,
                    c2 * (HEADS * P * 1024) + g * P,
                    [[1024 + 8, 16], [16 * 1024, HEADS * A], [1, A]],
                )
                dbase = e_st[g * 16 : (g + 1) * 16, :]
                dst = bass.AP(
                    dbase.tensor, dbase.offset,
                    [list(dbase.ap[0]), [A, HEADS * A], [1, A]],
                )
                nc.gpsimd.dma_start(dst, src_ap)
            # zero self-attention entries e[k,i,i]
            diag = bass.AP(
                e_st.tensor,
                e_st[:].offset,
                [list(e_st[:].ap[0]), [A * A, HEADS], [A + 1, A]],
            )
            nc.vector.memset(diag, 0.0)
            # Z and 1/Z
            z_t = scr.tile([P, HEADS * A], f32, tag="z_t")  # [(k,i)]
            nc.vector.tensor_reduce(
                out=z_t[:],
                in_=e_st[:].rearrange("p (ki j) -> p ki j", j=A),
                axis=AX.X,
                op=FP.add,
            )
            rz = scr.tile([P, HEADS * A], f32, tag="rz")
            nc.vector.reciprocal(rz[:], z_t[:])
            # wexp[(k,i,j,s=2)] = e * rz (softmax weight, dsub-expanded x2)
            wexp = scr.tile([P, HEADS * A * A * 2], f16, tag="wexp")
            nc.vector.tensor_tensor(
                out=bass.AP(wexp.tensor, wexp[:].offset,
                            [list(wexp[:].ap[0]), [16, 32], [2, A], [1, 2]]),
                in0=bass.AP(e_st.tensor, e_st[:].offset,
                            [list(e_st[:].ap[0]), [8, 32], [1, A], [0, 2]]),
                in1=bass.AP(rz.tensor, rz[:].offset,
                            [list(rz[:].ap[0]), [1, 32], [0, A], [0, 2]]),
                op=FP.mult,
            )

            # qvals: values[i,(k,d)] = sum_j wexp[k,i,j] * vals[j,(k,d)]
            if resident_values:
                values4 = vres5[:, c]
            else:
                values_all = chunkp.tile([P, A * H], f16, tag="values_all")
                values4 = values_all[:].rearrange(
                    "p (i k d) -> p i k d", i=A, k=HEADS
                )
            scaled = scr.tile([P, A * A * D], f16, tag="scaled")  # (j,i,d) per k
            for k in range(HEADS):
                for i in range(A):
                    # scaled[:, j, i, d] = vals[:, j, k, d] * wexp[:, k, i, j, (s)]
                    nc.vector.tensor_tensor(
                        out=bass.AP(scaled.tensor, scaled[:].offset + i * D,
                                    [list(scaled[:].ap[0]), [A * D, A], [2, 64], [1, 2]]),
                        in0=bass.AP(vals_all.tensor, vals_all[:].offset + k * D,
                                    [list(vals_all[:].ap[0]), [H, A], [2, 64], [1, 2]]),
                        in1=bass.AP(wexp.tensor, wexp[:].offset + k * 128 + i * 16,
                                    [list(wexp[:].ap[0]), [2, A], [0, 64], [1, 2]]),
                        op=FP.mult,
                    )
                # tree-add over j, in place in `scaled`
                nc.vector.tensor_tensor(
                    out=scaled[:, : 4 * A * D], in0=scaled[:, : 4 * A * D],
                    in1=scaled[:, 4 * A * D :], op=FP.add,
                )
                nc.vector.tensor_tensor(
                    out=scaled[:, : 2 * A * D], in0=scaled[:, : 2 * A * D],
                    in1=scaled[:, 2 * A * D : 4 * A * D], op=FP.add,
                )
                nc.vector.tensor_tensor(
                    out=values4[:, :, k, :],
                    in0=scaled[:, : A * D].rearrange("p (i d) -> p i d", i=A),
                    in1=scaled[:, A * D : 2 * A * D].rearrange("p (i d) -> p i d", i=A),
                    op=FP.add,
                )

            # values stats for phase-B LN (sum + sum of squares per agent)
            nc.vector.tensor_reduce(
                out=bass.AP(vsum.tensor, vsum[:].offset + c,
                            [list(vsum[:].ap[0]), [NCH, A]]),
                in_=values_all[:].rearrange("p (i h) -> p i h", i=A),
                axis=AX.X,
                op=FP.add,
            )
            for i in range(A):
                sqv2 = work.tile([P, H], f32, tag="sqv")
                nc.scalar.activation(
                    sqv2[:], values_all[:, i * H : (i + 1) * H], AF.Square,
                    accum_out=vasq[:, i * NCH + c : i * NCH + c + 1],
                )

            nc.sync.dma_start(
                vals_d[:, b0 : b0 + P, :].rearrange("a b h -> b a h"),
                values_all[:].rearrange("p (a h) -> p a h", a=A),
            )


        for _sc in range(NSC):
            emit_phase_b(_sc)

    nc.compile()
    return nc


_CACHE = {}


def kernel(**inputs):
    states = np.asarray(inputs["states"], np.float32)
    actions = np.asarray(inputs["actions"], np.float32)
    Ws_s = np.asarray(inputs["Ws_s"], np.float32)
    bs_s = np.asarray(inputs["bs_s"], np.float32)
    Ws_sa = np.asarray(inputs["Ws_sa"], np.float32)
    bs_sa = np.asarray(inputs["bs_sa"], np.float32)
    Wk = np.asarray(inputs["Wk"], np.float32)
    Wv = np.asarray(inputs["Wv"], np.float32)
    Wsel = np.asarray(inputs["Wsel"], np.float32)
    bsel = np.asarray(inputs["bsel"], np.float32)
    Wc1 = np.asarray(inputs["Wc1"], np.float32)
    bc1 = np.asarray(inputs["bc1"], np.float32)
    Wc2 = np.asarray(inputs["Wc2"], np.float32)
    bc2 = np.asarray(inputs["bc2"], np.float32)

    nonzero_bias = bool(
        np.any(bs_s) or np.any(bs_sa) or np.any(bsel)
    )
    if nonzero_bias not in _CACHE:
        _CACHE[nonzero_bias] = build(nonzero_bias)
    nc = _CACHE[nonzero_bias]

    # host-side weight prep (shared across cores)
    # Wk/Wv/Wsel [HEADS, H, D] -> [H, (head, d)] fp16
    wk_f = np.ascontiguousarray(Wk.transpose(1, 0, 2).reshape(H, H)).astype(np.float16)
    wv_f = np.ascontiguousarray(Wv.transpose(1, 0, 2).reshape(H, H)).astype(np.float16)
    wsel_f = np.ascontiguousarray(Wsel.transpose(1, 0, 2).reshape(H, H)).astype(np.float16)
    shared = {
        "ws_s": Ws_s.astype(np.float16),
        "ws_sa": Ws_sa.astype(np.float16),
        "wk": wk_f,
        "wv": wv_f,
        "wsel": wsel_f,
        "wc1": Wc1.astype(np.float16),
        "wc2": Wc2.astype(np.float16),
        "bs_s": bs_s.astype(np.float16),
        "bs_sa": bs_sa.astype(np.float16),
        "bsel": bsel.astype(np.float16),
        "bc1": bc1,
        "bc2": bc2,
    }
    in_maps = []
    for core in range(NCORES):
        sl = slice(core * BL, (core + 1) * BL)
        m = dict(shared)
        m["states"] = np.ascontiguousarray(states[:, sl, :])
        m["actions"] = np.ascontiguousarray(actions[:, sl, :])
        in_maps.append(m)

    res = run_bass_kernel_spmd(nc, in_maps, core_ids=list(range(NCORES)))
    # gather: q[a, b] = allqs[a, idx[a, b], b]
    idx = np.argmax(actions, axis=-1)  # [A, B]
    q = np.empty((A, B, 1), np.float32)
    for core in range(NCORES):
        allqs = res.results[core]["allqs"]  # [A, ADIM, BL]
        sl = slice(core * BL, (core + 1) * BL)
        ii = idx[:, sl]
        q[:, sl, 0] = np.take_along_axis(
            allqs, ii[:, None, :], axis=1
        )[:, 0, :]
    return q


def _install_ntff_hook():
    """The agent image's antenv lacks axon_hooks; synthesize it so
    run_bass_kernel_spmd(trace=True) can capture NTFF profiles."""
    import types
    import antenv

    if "antenv.axon_hooks" in sys.modules:
        return
    mod = types.ModuleType("antenv.axon_hooks")
    mod._hook = None

    def set_axon_ntff_profile_hook(h):
        mod._hook = h

    def get_axon_ntff_profile_hook():
        return mod._hook

    mod.set_axon_ntff_profile_hook = set_axon_ntff_profile_hook
    mod.get_axon_ntff_profile_hook = get_axon_ntff_profile_hook
    sys.modules["antenv.axon_hooks"] = mod
    antenv.axon_hooks = mod
    sys.path.insert(0, "/root/.axon_site")
    from trn_agent_boot.trn_boot import _ntff_profile_via_ctypes

    hook = _ntff_profile_via_ctypes("/opt/axon/libaxon_pjrt.so")
    if hook is not None:
        set_axon_ntff_profile_hook(hook)


def profile_run(inputs):
    """Traced run returning HW exec time in ns (max across cores)."""
    import os
    os.environ["BASS_PERFETTO_PROFILE_ALL_CORES"] = "1"
    _install_ntff_hook()
    states = np.asarray(inputs["states"], np.float32)
    actions = np.asarray(inputs["actions"], np.float32)
    nonzero_bias = bool(
        np.any(inputs["bs_s"]) or np.any(inputs["bs_sa"]) or np.any(inputs["bsel"])
    )
    if nonzero_bias not in _CACHE:
        _CACHE[nonzero_bias] = build(nonzero_bias)
    nc = _CACHE[nonzero_bias]
    wk_f = np.ascontiguousarray(np.asarray(inputs["Wk"], np.float32).transpose(1, 0, 2).reshape(H, H)).astype(np.float16)
    wv_f = np.ascontiguousarray(np.asarray(inputs["Wv"], np.float32).transpose(1, 0, 2).reshape(H, H)).astype(np.float16)
    wsel_f = np.ascontiguousarray(np.asarray(inputs["Wsel"], np.float32).transpose(1, 0, 2).reshape(H, H)).astype(np.float16)
    shared = {
        "ws_s": np.asarray(inputs["Ws_s"], np.float32).astype(np.float16),
        "ws_sa": np.asarray(inputs["Ws_sa"], np.float32).astype(np.float16),
        "wk": wk_f, "wv": wv_f, "wsel": wsel_f,
        "wc1": np.asarray(inputs["Wc1"], np.float32).astype(np.float16),
        "wc2": np.asarray(inputs["Wc2"], np.float32).astype(np.float16),
        "bs_s": np.asarray(inputs["bs_s"], np.float16),
        "bs_sa": np.asarray(inputs["bs_sa"], np.float16),
        "bsel": np.asarray(inputs["bsel"], np.float16),
        "bc1": np.asarray(inputs["bc1"], np.float32),
        "bc2": np.asarray(inputs["bc2"], np.float32),
    }
    in_maps = []
    for core in range(NCORES):
        sl = slice(core * BL, (core + 1) * BL)
        m = dict(shared)
        m["states"] = np.ascontiguousarray(states[:, sl, :])
        m["actions"] = np.ascontiguousarray(actions[:, sl, :])
        in_maps.append(m)
    res = run_bass_kernel_spmd(
        nc, in_maps, core_ids=list(range(NCORES)), trace=True,
        tmpdir="/tmp/prof", trace_cores=[0],
    )
    print("mean exec:", res.mean_exec_time_ns, "max core:", res.max_exec_time_core_id)
    return res.exec_time_ns

